# revision 8
# baseline (speedup 1.0000x reference)
"""CLIP attention Bass/Tile kernel for TRN2, v2. One core = one batch element.

Changes vs v1:
  - q/k projections: fp8e4m3 DoubleRow matmuls (2 contraction rows per
    partition -> 4 matmuls instead of 8, each at 0.5 cycles/row).
  - v projection: 3-group fp8 DoubleRow with host-side error feedback
    (x8*W8 + xr*W8 + x8*Wr) so its error stays ~3e-3 while costing 12
    DoubleRow matmuls vs 8 fp32r ones.
  - probs/v' stored fp16 (better mantissa than bf16, same PE cost).
  - o-proj split into pair-halves 0-3 / 4-7; first half runs mid-stream,
    halves summed on DVE, so only half the o-proj remains as tail.
  - exp applies scale=SCALE/64^2 on ACT (weights are pre-scaled by 64 to
    center fp8; biases pre-scaled by 64; denominator column is 64.0 so
    the PV numerator/denominator ratio is exact).

Device-side layout (per core):
  inputs (DRAM):
    xf  [128 p, 4 ktp, 2 t, 1024 s] fp8:  x8[s, (2ktp+t)*128+p]
    xr  same shape fp8: fp8(x - x8)
    wqf [128 p, 8 pr, 4 ktp, 2 t, 128 m] fp8: 64*Wq[pr*128+m, (2ktp+t)*128+p]
    wkf likewise
    wvf [128 p, 4 ktp, 2 t, 1024 n] fp8: W8 of 64*Wv[n, (2ktp+t)*128+p]
    wvr same shape fp8: fp8(64*Wv.T - W8)
    wo  [1024, 1024] f32: Wo.T     bq64/bk64/bv64 [1024] f32 = 64*b; bo [1024]
  output: y [1024 s, 1024 d] f32

Per core:
  vv[p, jt, h*65+e] fp16 = 64*v[jt*128+p, h*64+e] + bias; e=64 -> 64.0
  per head-pair p (8):
    qt/kt [128 d, 1024 s] f32r = 64*(q/k incl bias)
    ST_h [128 j, 1024 i] psum = kt_h.T @ qt_h   (fp32r, 64-contract)
    ex = exp(ST * SCALE/4096) -> fp16           (ACT)
    acc[65, 512] += vv_h.T @ ex                 (fp16, row 64 = 64*denom)
    outT = acc[0:64] * (1/acc[64])              (recip + DMA bcast + Pool mul)
  y = outT.T @ woT + bo, contraction over pairs 0-3 mid-stream + 4-7 tail.
"""

import sys

sys.path.insert(0, "/opt/trn_rl_repo")

from contextlib import ExitStack
from itertools import chain as _chain

import concourse.bass as bass
import concourse.mybir as mybir
import concourse.tile as tile
from concourse import bacc

F32 = mybir.dt.float32
F32R = mybir.dt.float32r
FP8 = mybir.dt.float8e4
FP16 = mybir.dt.float16
AF = mybir.ActivationFunctionType
DR = mybir.MatmulPerfMode.DoubleRow

D = 1024
S = 1024
H = 16
DH = 64
P = 128
KTP = 4  # DoubleRow k-tile pairs over the 1024 contraction
NPAIR = 8
SCALE = DH ** -0.5
EXP_SCALE = SCALE / 4096.0  # undoes the 64x on each of q and k


def build_nc(reps=1):
    nc = bacc.Bacc("TRN2", target_bir_lowering=False, debug=False, num_devices=1)
    _state["last_nc"] = nc

    xf_d = nc.dram_tensor("xf", [P, KTP, 2, S], FP8, kind="ExternalInput").ap()
    xr_d = nc.dram_tensor("xr", [P, KTP, 2, S], FP8, kind="ExternalInput").ap()
    wqf_d = nc.dram_tensor("wqf", [P, NPAIR, KTP, 2, P], FP8, kind="ExternalInput").ap()
    wkf_d = nc.dram_tensor("wkf", [P, NPAIR, KTP, 2, P], FP8, kind="ExternalInput").ap()
    wvf_d = nc.dram_tensor("wvf", [P, KTP, 2, D], FP8, kind="ExternalInput").ap()
    wvr_d = nc.dram_tensor("wvr", [P, KTP, 2, D], FP8, kind="ExternalInput").ap()
    wo_d = nc.dram_tensor("wo", [D, D], F32R, kind="ExternalInput").ap()
    bq_d = nc.dram_tensor("bq64", [D], F32, kind="ExternalInput").ap()
    bk_d = nc.dram_tensor("bk64", [D], F32, kind="ExternalInput").ap()
    bv_d = nc.dram_tensor("bv64", [D], F32, kind="ExternalInput").ap()
    bo_d = nc.dram_tensor("bo", [D], F32, kind="ExternalInput").ap()
    y_d = nc.dram_tensor("y", [S, D], F32, kind="ExternalOutput").ap()

    with tile.TileContext(nc) as tc:
        for rep in range(reps):
            _emit(nc, tc, rep, xf_d, xr_d, wqf_d, wkf_d, wvf_d, wvr_d, wo_d,
                  bq_d, bk_d, bv_d, bo_d, y_d)

    nc.compile()
    return nc


def _emit(nc, tc, rep, xf_d, xr_d, wqf_d, wkf_d, wvf_d, wvr_d, wo_d,
          bq_d, bk_d, bv_d, bo_d, y_d):
    R = f"r{rep}_"
    with ExitStack() as ctx:
        consts = ctx.enter_context(tc.tile_pool(name=R + "consts", bufs=1))
        big = ctx.enter_context(tc.tile_pool(name=R + "big", bufs=1))
        qk_pool = ctx.enter_context(tc.tile_pool(name=R + "qk", bufs=2))
        qf_pool = ctx.enter_context(tc.tile_pool(name=R + "qf", bufs=2))
        wqk_pool = ctx.enter_context(tc.tile_pool(name=R + "wqk", bufs=2))
        exa_pool = ctx.enter_context(tc.tile_pool(name=R + "exa", bufs=3))
        exb_pool = ctx.enter_context(tc.tile_pool(name=R + "exb", bufs=9))
        small = ctx.enter_context(tc.tile_pool(name=R + "small", bufs=5))
        wo_pool = ctx.enter_context(tc.tile_pool(name=R + "wo", bufs=10))
        y_pool = ctx.enter_context(tc.tile_pool(name=R + "yout", bufs=3))
        ps_proj = ctx.enter_context(tc.tile_pool(name=R + "ps_proj", bufs=2, space="PSUM"))
        ps_st = ctx.enter_context(tc.tile_pool(name=R + "ps_st", bufs=2, space="PSUM"))
        ps_acc = ctx.enter_context(tc.tile_pool(name=R + "ps_acc", bufs=2, space="PSUM"))

        # ---- PE warmup: dependency-free matmuls ramp the clock to 2.4GHz
        wup = consts.tile([P, 2, 512], FP8, name=R + "wup")
        nc.vector.memset(wup[:].bitcast(mybir.dt.uint8), 0)
        pw = ps_proj.tile([P, 512], F32, tag="ps_proj", name=R + "warm")
        for i in range(20):
            nc.tensor.matmul(pw[:], wup[:, :, 0:128], wup[:],
                             start=(i == 0), stop=(i == 19), perf_mode=DR)
        # the BIR verifier requires a reader for every psum tensor
        wupf = consts.tile([1, 1], F32, name=R + "wupf")
        nc.vector.tensor_copy(wupf[:], pw[0:1, 0:1])

        # ---- critical-path loads first, on the SP hwdge queue ----
        wq_p0 = wqk_pool.tile([P, KTP, 2, P], FP8, tag="wq", name=R + "wqp0")
        wk_p0 = wqk_pool.tile([P, KTP, 2, P], FP8, tag="wk", name=R + "wkp0")
        nc.sync.dma_start(wq_p0[:], wqf_d[:, 0])
        nc.sync.dma_start(wk_p0[:], wkf_d[:, 0])
        xf = big.tile([P, KTP, 2, S], FP8, name=R + "xf")
        nc.sync.dma_start(xf[:], xf_d)

        # ---- remaining loads, priority order on the same SP queue; the v-path
        # tensors arrive in halves so v-proj st0 can start early ----
        bqt = consts.tile([P, NPAIR], F32, name=R + "bqt")  # 64*bq[m*128+p] at [p, m]
        bkt = consts.tile([P, NPAIR], F32, name=R + "bkt")
        nc.sync.dma_start(bqt[:], bq_d.rearrange("(m p) -> p m", p=P))
        nc.sync.dma_start(bkt[:], bk_d.rearrange("(m p) -> p m", p=P))
        wvf = big.tile([P, KTP, 2, D], FP8, name=R + "wvf")
        xr = big.tile([P, KTP, 2, S], FP8, name=R + "xr")
        wvr = big.tile([P, KTP, 2, D], FP8, name=R + "wvr")
        bv_b = consts.tile([P, D], F32, name=R + "bv_b")  # bcast along partitions
        nc.sync.dma_start(wvf[:, :, :, 0:512], wvf_d[:, :, :, 0:512])
        nc.sync.dma_start(xr[:, :, :, 0:512], xr_d[:, :, :, 0:512])
        nc.sync.dma_start(wvr[:, :, :, 0:512], wvr_d[:, :, :, 0:512])
        nc.sync.dma_start(bv_b[:, 0:512], bass.AP(bv_d.tensor, bv_d.offset, [[0, P], [1, 512]]))
        nc.sync.dma_start(wvf[:, :, :, 512:1024], wvf_d[:, :, :, 512:1024])
        nc.sync.dma_start(xr[:, :, :, 512:1024], xr_d[:, :, :, 512:1024])
        nc.sync.dma_start(wvr[:, :, :, 512:1024], wvr_d[:, :, :, 512:1024])
        nc.sync.dma_start(bv_b[:, 512:1024], bass.AP(bv_d.tensor, bv_d.offset + 512, [[0, P], [1, 512]]))
        bo_b = consts.tile([P, D], F32, name=R + "bo_b")
        nc.sync.dma_start(bo_b[:], bass.AP(bo_d.tensor, bo_d.offset, [[0, P], [1, D]]))

        # ---- augmented v' ----
        # vv[p, jt, h*65+e] = 64*v[jt*128+p, h*64+e]; e=64 -> 64.0
        vv = big.tile([P, NPAIR, H * 65], FP16, name=R + "vv")
        for jt in range(NPAIR):
            nc.vector.memset(
                vv[:, jt, :].rearrange("p (h e) -> p h e", e=65)[:, :, 64:65],
                64.0,
            )

        # ---- outT accumulator + o-proj half accumulator ----
        ot = big.tile([P, NPAIR, S], F32R, name=R + "ot")  # d = pair*128 + p
        y_acc = big.tile([P, 16, 512], F32, name=R + "y_acc")  # (nh*8+st)

        def vproj_steps():
            for st in range(NPAIR):  # st-major so vv[jt] completes just in time
                for nh in range(2):
                    ps = ps_proj.tile([P, 512], F32, tag="ps_proj",
                                      name=f"{R}psv{nh}_{st}")
                    first = True
                    for xa, wb in ((xf, wvf), (xr, wvf), (xf, wvr)):
                        for ktp in range(KTP):
                            nc.tensor.matmul(
                                ps[:],
                                xa[:, ktp, :, st * P : (st + 1) * P],
                                wb[:, ktp, :, nh * 512 : (nh + 1) * 512],
                                start=first,
                                stop=(xa is xf and wb is wvr and ktp == KTP - 1),
                                perf_mode=DR,
                            )
                            first = False
                    vv_dst = vv[:, st, nh * 520 : (nh + 1) * 520].rearrange(
                        "p (h e) -> p h e", e=65
                    )[:, :, 0:64]
                    nc.vector.tensor_add(
                        vv_dst,
                        ps[:].rearrange("p (h e) -> p h e", e=64),
                        bv_b[:, nh * 512 : (nh + 1) * 512].rearrange(
                            "p (h e) -> p h e", e=64
                        ),
                    )
                    yield

        def emit_qkproj_dma(p):
            wq_p = wqk_pool.tile([P, KTP, 2, P], FP8, tag="wq", name=f"{R}wqp{p}")
            wk_p = wqk_pool.tile([P, KTP, 2, P], FP8, tag="wk", name=f"{R}wkp{p}")
            nc.sync.dma_start(wq_p[:], wqf_d[:, p])
            nc.sync.dma_start(wk_p[:], wkf_d[:, p])
            return wq_p, wk_p

        def make_qk_tiles(p):
            qt = qk_pool.tile([P, S], FP8, tag="qt", name=f"{R}qt{p}")
            kt = qk_pool.tile([P, S], FP8, tag="kt", name=f"{R}kt{p}")
            if p == 0:
                return qt, kt, None, None  # pair 0 needs no folded tiles
            qf = qf_pool.tile([64, 2, S], FP8, tag="qf", name=f"{R}qf{p}")
            kf = qf_pool.tile([64, 2, S], FP8, tag="kf", name=f"{R}kf{p}")
            return qt, kt, qf, kf

        def emit_qkproj_steps(p, wq_p, wk_p, qkt):
            """Generator yielding after each matmul group; 4 groups total."""
            qt, kt, qf, kf = qkt
            dma_eng = None if p == 0 else nc.sync
            for wt, outt, foldt, biast, wn in (
                (wq_p, qt, qf, bqt, "q"), (wk_p, kt, kf, bkt, "k")
            ):
                for ih in range(2):
                    ps = ps_proj.tile([P, 512], F32, tag="ps_proj",
                                      name=f"{R}psqk{p}_{wn}_{ih}")
                    for ktp in range(KTP):
                        nc.tensor.matmul(
                            ps[:],
                            wt[:, ktp],
                            xf[:, ktp, :, ih * 512 : (ih + 1) * 512],
                            start=(ktp == 0),
                            stop=(ktp == KTP - 1),
                            perf_mode=DR,
                        )
                    nc.vector.tensor_scalar_add(
                        outt[:, ih * 512 : (ih + 1) * 512], ps[:], biast[:, p : p + 1]
                    )
                    if ih == 1 and dma_eng is not None:
                        # fold [128 d, s] -> [32 pp, 2 t, s] per head half:
                        # foldt[hl*32+pp, t, s] = outt[hl*64 + t*32 + pp, s]
                        for hl in range(2):
                            for t in range(2):
                                dma_eng.dma_start(
                                    foldt[hl * 32 : (hl + 1) * 32, t, :],
                                    outt[hl * 64 + t * 32 : hl * 64 + (t + 1) * 32, :],
                                )
                    yield

        def emit_scores(p, qkt, jt):
            qt, kt, qf, kf = qkt
            sts = [
                ps_st.tile([P, S], F32, tag="st", name=f"{R}st{p}_{jt}_{hl}")
                for hl in range(2)
            ]
            for ih in range(2):
                for hl in range(2):  # alternate row groups for PE concurrency
                    if p == 0:
                        # pair 0 skips the fold (its DMAs would queue behind
                        # the v-path bulk): plain fp8 matmul, 64-contract
                        b = hl * 64
                        nc.tensor.matmul(
                            sts[hl][:, ih * 512 : (ih + 1) * 512],
                            kt[b : b + 64, jt * P : (jt + 1) * P],
                            qt[b : b + 64, ih * 512 : (ih + 1) * 512],
                            start=True,
                            stop=True,
                        )
                    else:
                        b = hl * 32
                        nc.tensor.matmul(
                            sts[hl][:, ih * 512 : (ih + 1) * 512],
                            kf[b : b + 32, :, jt * P : (jt + 1) * P],
                            qf[b : b + 32, :, ih * 512 : (ih + 1) * 512],
                            start=True,
                            stop=True,
                            perf_mode=DR,
                        )
            return sts

        def emit_pv(h, jt, ex, accs):
            for ih in range(2):
                nc.tensor.matmul(
                    accs[ih][:],
                    vv[:, jt, h * 65 : (h + 1) * 65],
                    ex[:, ih * 512 : (ih + 1) * 512],
                    start=(jt == 0),
                    stop=(jt == NPAIR - 1),
                )

        def acc_copy(p, hl, ih, acc):
            # reciprocal straight from psum so it doesn't chain behind the copy
            rec = small.tile([1, 512], F32, tag="rec", name=f"{R}rec{p}_{hl}_{ih}")
            nc.vector.reciprocal(rec[:], acc[64:65, :])
            cp = small.tile([64, 512], F32, tag="cp", name=f"{R}cp{p}_{hl}_{ih}")
            nc.vector.tensor_copy(cp[:], acc[0:64, :])
            return cp, rec

        def normalize_bcast(p, hl, ih, cprec):
            rb = small.tile([64, 512], F32, tag="rb", name=f"{R}rb{p}_{hl}_{ih}")
            nc.gpsimd.partition_broadcast(rb[:], cprec[1][:], channels=64)
            return rb

        def normalize_mul(p, hl, ih, cprec, rb):
            b = hl * 64
            eng = nc.vector if (p >= 7 or p % 2 == 1) else nc.gpsimd
            eng.tensor_mul(
                ot[b : b + 64, p, ih * 512 : (ih + 1) * 512], cprec[0][:], rb[:]
            )

        wo_r = wo_d.rearrange("(k p) (nh n) -> p k nh n", p=P, n=512)
        y_r = y_d.rearrange("(st p) n -> st p n", p=P)

        OPROJ_QUARTERS = ((0, 1), (2, 3), (4, 5), (6, 7))

        def emit_wo_dma(quarter):
            wots = {}
            for nh in range(2):
                for i, pr in enumerate(OPROJ_QUARTERS[quarter]):
                    wot = wo_pool.tile([P, 512], F32R, tag="wot",
                                       name=f"{R}wot{quarter}_{nh}_{i}")
                    nc.scalar.dma_start(wot[:], wo_r[:, pr, nh, :])
                    wots[nh, i] = wot
            return wots

        def oproj_steps(quarter, wots):
            prs = OPROJ_QUARTERS[quarter]
            for nh in range(2):
                for st in range(8):
                    ps = ps_proj.tile([P, 512], F32, tag="ps_proj",
                                      name=f"{R}psy{quarter}_{nh}_{st}")
                    for i, pr in enumerate(prs):
                        nc.tensor.matmul(
                            ps[:],
                            ot[:, pr, st * P : (st + 1) * P],
                            wots[nh, i][:],
                            start=(i == 0),
                            stop=(i == len(prs) - 1),
                        )
                    sl = y_acc[:, nh * 8 + st, :]
                    if quarter == 0:
                        nc.vector.tensor_add(
                            sl, ps[:], bo_b[:, nh * 512 : (nh + 1) * 512],
                        )
                    elif quarter < 3:
                        nc.vector.tensor_add(sl, ps[:], sl)
                    else:
                        yt = y_pool.tile([P, 512], F32, tag="yt",
                                         name=f"{R}yt{nh}_{st}")
                        nc.vector.tensor_add(yt[:], ps[:], y_acc[:, nh * 8 + st, :])
                        nc.sync.dma_start(y_r[st, :, nh * 512 : (nh + 1) * 512], yt[:])
                    yield

        def emit_norms(p, cpsA, cpsB):
            order = [(0, 0), (1, 0), (0, 1), (1, 1)]  # (hl, ih), ih0 first
            cps = {0: cpsA, 1: cpsB}
            rbs = {}
            for hl, ih in order:
                rbs[hl, ih] = normalize_bcast(p, hl, ih, cps[hl][ih])
            for hl, ih in order:
                normalize_mul(p, hl, ih, cps[hl][ih], rbs[hl, ih])

        def emit_attention(p, qkt, bg, n_bg, delay_bg=False):
            """Scores+exp+PV-A for pair p (PV-B inline only for the last pair).
            Returns (exBs, cpsA) for the deferred epilogue."""
            hA, hB = 2 * p, 2 * p + 1
            last = p == NPAIR - 1
            accA = [
                ps_acc.tile([65, 512], F32, tag="acc", name=f"{R}accA_{p}_{ih}")
                for ih in range(2)
            ]
            accB = None
            pulled = 0
            exAs = []
            exBs = []
            for jt in range(NPAIR):
                stA, stB = emit_scores(p, qkt, jt)
                exa = exa_pool.tile([P, S], FP16, tag="exa", name=f"{R}exa{p}_{jt}")
                nc.scalar.activation(exa[:], stA[:], AF.Exp, scale=EXP_SCALE)
                exb = exb_pool.tile([P, S], FP16, tag="exb", name=f"{R}exb{p}_{jt}")
                nc.scalar.activation(exb[:], stB[:], AF.Exp, scale=EXP_SCALE)
                exAs.append(exa)
                exBs.append(exb)
                if delay_bg:
                    # the first 5 steps are the previous pair's epilogue: pull
                    # them promptly (exb/acc ring slots depend on them), delay
                    # only the rest (o-proj quarters that trail the norms)
                    head = min(5, 2 * (jt + 1))
                    rest = max(0, n_bg - 5)
                    want = head + -(-rest * max(0, jt - 1) // (NPAIR - 2))
                else:
                    want = -(-n_bg * (jt + 1) // NPAIR)  # ceil pacing
                while pulled < want and next(bg, "END") != "END":
                    pulled += 1
                if jt >= 1:  # one-jt lag so PV never waits on ACT
                    emit_pv(hA, jt - 1, exAs[jt - 1], accA)
                if last and jt >= 2:
                    # last pair: B accumulates inline, borrowing the ps_proj
                    # ring (no projections left to need it)
                    if accB is None:
                        accB = [
                            ps_proj.tile([65, 512], F32, tag="ps_proj",
                                         name=f"{R}accB_{p}_{ih}")
                            for ih in range(2)
                        ]
                    emit_pv(hB, jt - 2, exBs[jt - 2], accB)
            emit_pv(hA, NPAIR - 1, exAs[NPAIR - 1], accA)
            cpsA = [acc_copy(p, 0, ih, accA[ih]) for ih in range(2)]
            if last:
                emit_pv(hB, NPAIR - 2, exBs[NPAIR - 2], accB)
                emit_pv(hB, NPAIR - 1, exBs[NPAIR - 1], accB)
                cpsB = [acc_copy(p, 1, ih, accB[ih]) for ih in range(2)]
                emit_norms(p, cpsA, cpsB)
            return exBs, cpsA

        def epilogue_steps(p, exBs, cpsA):
            """Deferred PV-B + copies + norms for pair p, run during pair p+1."""
            hB = 2 * p + 1
            accB = [
                ps_acc.tile([65, 512], F32, tag="acc", name=f"{R}accB_{p}_{ih}")
                for ih in range(2)
            ]
            for jtp in range(4):
                emit_pv(hB, 2 * jtp, exBs[2 * jtp], accB)
                emit_pv(hB, 2 * jtp + 1, exBs[2 * jtp + 1], accB)
                yield
            cpsB = [acc_copy(p, 1, ih, accB[ih]) for ih in range(2)]
            emit_norms(p, cpsA, cpsB)
            yield

        # ---- main pipeline ----
        # Persistent background queue: generators are appended as their data
        # becomes available and drained with per-pair budgets, so heavy items
        # (v-proj, o-proj quarters) spread across pairs.
        from collections import deque

        bgq = deque()  # (name, generator) FIFO

        def pull():
            while bgq:
                if next(bgq[0][1], "END") == "END":
                    bgq.popleft()
                else:
                    return True
            return False

        def drain(name):
            while any(n == name for n, _ in bgq):
                if not pull():
                    break

        class _Puller:
            def __iter__(self):
                return self

            def __next__(self):
                if not pull():
                    raise StopIteration
                return True

        puller = _Puller()

        qkt = make_qk_tiles(0)
        for _ in emit_qkproj_steps(0, wq_p0, wk_p0, qkt):
            pass
        cur = qkt
        wots = {}
        epi = None
        budgets = {0: 20, 1: 9, 2: 17, 3: 17, 4: 17, 5: 17, 6: 21, 7: 5}
        for p in range(NPAIR):
            if p in (1, 3, 5):
                wots[(p - 1) // 2] = emit_wo_dma((p - 1) // 2)
            if p == 7:
                wots[3] = emit_wo_dma(3)
            if p == 0:
                bgq.append(("v", vproj_steps()))
            if epi is not None:
                bgq.appendleft(("epi", epi))
            if p + 1 < NPAIR:
                wq_n, wk_n = emit_qkproj_dma(p + 1)
                qkt_n = make_qk_tiles(p + 1)
                bgq.append(("qk", emit_qkproj_steps(p + 1, wq_n, wk_n, qkt_n)))
            else:
                qkt_n = None
            if p in (2, 4, 6):
                bgq.append(("oproj", oproj_steps(p // 2 - 1, wots[p // 2 - 1])))
            exBs, cpsA = emit_attention(
                p, cur, puller, budgets[p], delay_bg=(p >= 2)
            )
            # correctness drains: next pair's q/k must exist before its scores;
            # the epilogue must finish within this pair.
            drain("epi")
            drain("qk")
            if p == 6:
                drain("oproj")  # pair 7 borrows the ps_proj ring for PV-B
            epi = None if p == NPAIR - 1 else epilogue_steps(p, exBs, cpsA)
            cur = qkt_n
        while pull():
            pass
        for _ in oproj_steps(3, wots[3]):
            pass


# ---------------------------------------------------------------------------
# Host-side entry point: full inputs in, full output out.
# Data parallel: core b computes batch element b.
# ---------------------------------------------------------------------------

import numpy as np

NP_FP8 = mybir.dt.np(FP8)
_state = {}


def _get_nc():
    if "nc" not in _state:
        _state["nc"] = build_nc()
    return _state["nc"]


def _fold(a):
    # [1024 d_in, n] -> [128 p, ktp, t, n] with d_in = (2ktp+t)*128+p
    n = a.shape[1]
    return np.ascontiguousarray(a.reshape(KTP, 2, P, n).transpose(2, 0, 1, 3))


def _q8(a):
    return a.astype(NP_FP8)


def prep_in_maps(hidden_states, Wq, bq, Wk, bk, Wv, bv, Wo, bo):
    hidden_states = np.asarray(hidden_states, dtype=np.float32)
    Wq = np.asarray(Wq, dtype=np.float32)
    Wk = np.asarray(Wk, dtype=np.float32)
    Wv = np.asarray(Wv, dtype=np.float32)
    Wo = np.asarray(Wo, dtype=np.float32)
    bq = np.asarray(bq, dtype=np.float32)
    bk = np.asarray(bk, dtype=np.float32)
    bv = np.asarray(bv, dtype=np.float32)
    bo = np.asarray(bo, dtype=np.float32)

    wq64 = _fold(64.0 * Wq.T)
    wqf = _q8(wq64).reshape(P, KTP, 2, NPAIR, P).transpose(0, 3, 1, 2, 4)
    wqf = np.ascontiguousarray(wqf)
    wk64 = _fold(64.0 * Wk.T)
    wkf = _q8(wk64).reshape(P, KTP, 2, NPAIR, P).transpose(0, 3, 1, 2, 4)
    wkf = np.ascontiguousarray(wkf)
    wv64 = _fold(64.0 * Wv.T)
    wvf = _q8(wv64)
    wvr = _q8(wv64 - wvf.astype(np.float32))
    wo = np.ascontiguousarray(Wo.T)
    bq64 = 64.0 * bq
    bk64 = 64.0 * bk
    bv64 = 64.0 * bv

    nb = hidden_states.shape[0]
    in_maps = []
    for b in range(nb):
        x = _fold(np.ascontiguousarray(hidden_states[b].T))
        xf = _q8(x)
        xr = _q8(x - xf.astype(np.float32))
        in_maps.append(
            {
                "xf": xf, "xr": xr, "wqf": wqf, "wkf": wkf,
                "wvf": wvf, "wvr": wvr, "wo": wo,
                "bq64": bq64, "bk64": bk64, "bv64": bv64, "bo": bo,
            }
        )
    return in_maps


def kernel(hidden_states, Wq, bq, Wk, bk, Wv, bv, Wo, bo):
    from concourse.bass_utils import run_bass_kernel_spmd

    in_maps = prep_in_maps(hidden_states, Wq, bq, Wk, bk, Wv, bv, Wo, bo)
    nc = _get_nc()
    res = run_bass_kernel_spmd(nc, in_maps, core_ids=list(range(len(in_maps))))
    return np.stack([res.results[b]["y"] for b in range(len(in_maps))]).astype(
        np.float32
    )
